# revision 26
# baseline (speedup 1.0000x reference)
"""AggBondModule kernel for Trainium2 (8 NeuronCores, SPMD edge-sharding).

out[e] = relu(concat(node_feat[src[e]], node_feat[dst[e]], edge_feat[e]) @ W + b)

Per core (50000 edges), all feature data is bf16 on device. Node rows are
fetched with the ANT `dma_gather` instruction in TRANSPOSE mode: each
2048-index gather lands feature-major ([128 feat partitions, 2048 edges])
directly, so no PE transposes or PSUM round-trips are needed. The gathers
are round-robined over the 4 SWDGE queues so all four Q7 core pairs
generate descriptors concurrently (descriptor generation on a single pair
is the baseline bottleneck).

Since gather indices are int16 (< 32768) and there are 50000 nodes, the
node table is addressed in two halves and the host pre-sorts each core's
edges into four classes by (src < 32768, dst < 32768); each 2048-edge
block gathers from a fixed half per operand with rebased indices. Pad
slots use index 0. The host un-permutes the output.

edge_feat is host-transposed per block ([128, 2048] bf16) so it DMAs
straight into feature-major tiles. Compute per 512-edge group: three bf16
matmuls (W chunks stationary, 512-edge moving dim) accumulate into PSUM;
ACT applies bias+ReLU on the PSUM->SBUF copy, emitting bf16. Output is
stored transposed per block and rearranged on the host.
"""

import sys

import numpy as np

sys.path.insert(0, "/opt/trn_rl_repo")

import ml_dtypes

BF16 = np.dtype(ml_dtypes.bfloat16)

P = 128
D = 128  # node/edge feature dim; also output dim
N_NODES = 50000
HALF = 32768  # int16-addressable node-table split
E_TOTAL = 400000
N_CORES = 8
C = 16  # 128-edge chunks per block
BLOCK = P * C  # 2048
# blocks per (src-half, dst-half) class: AA, AB, BA, BB
CLASS_BLOCKS = (11, 6, 6, 3)
E_LOC = E_TOTAL // N_CORES  # 50000
N_QUEUES = 4  # SWDGE queues (each maps to its own Q7 core pair)
SINGLE_PACKET = False
GATHER_CHUNK = 896  # idxs per dma_gather call in transpose mode
GATHER_CHUNK_NT = 2048  # idxs per dma_gather call in non-transpose mode
# Transpose-mode gathers land feature-major (no PE transposes) but their
# XBAR spray streams corrupt each other when gathers run concurrently on
# multiple SWDGE queues. Non-transpose gathers land edge-major (PE
# transposes + DVE evac needed) but are XBAR-free, so 4-queue concurrent
# descriptor generation is safe.
TRANSPOSE_GATHER = False


def build_program(
    n_nodes=N_NODES,
    half=HALF,
    c=C,
    class_blocks=CLASS_BLOCKS,
    num_devices=N_CORES,
):
    from concourse import bacc, mybir, tile

    block = P * c
    n_blocks = sum(class_blocks)
    n_groups = block // 512  # 512-edge matmul groups per block
    idx_words = block // 16
    f32 = mybir.dt.float32
    bf16 = mybir.dt.bfloat16
    i16 = mybir.dt.int16

    nc = bacc.Bacc(
        "TRN2",
        target_bir_lowering=False,
        debug=False,
        num_devices=num_devices,
        num_swdge_queues=N_QUEUES,
    )

    node_feat = nc.dram_tensor("node_feat", [n_nodes, D], bf16, kind="ExternalInput").ap()
    # edge_featT[b, f, j] = edge_feat[edge (b*block + j), f]  (host-prepared)
    edge_featT = nc.dram_tensor(
        "edge_featT", [n_blocks, P, block], bf16, kind="ExternalInput"
    ).ap()
    # W_r[f, k, j] = W[k*128 + f, j]  (host-prepared)
    w_dram = nc.dram_tensor("W", [P, 3, D], bf16, kind="ExternalInput").ap()
    b_dram = nc.dram_tensor("b", [D], f32, kind="ExternalInput").ap()
    # all blocks' wrapped indices, preloaded in one DMA:
    # idx_all[p, ((b*2+which)*idx_words + w)], which: 0=src, 1=dst
    idx_dram = nc.dram_tensor(
        "idx_all", [P, n_blocks * 2 * idx_words], i16, kind="ExternalInput"
    ).ap()
    if not TRANSPOSE_GATHER:
        ident_dram = nc.dram_tensor("ident", [P, P], bf16, kind="ExternalInput").ap()
    outT = nc.dram_tensor("outT", [n_blocks, P, block], bf16, kind="ExternalOutput").ap()

    nf_lo = node_feat[:half, :]
    nf_hi = node_feat[half:, :]

    # block -> (src_half_hi, dst_half_hi)
    block_cls = []
    for cls_i, nb in enumerate(class_blocks):
        block_cls += [(cls_i >> 1 & 1, cls_i & 1)] * nb

    gather_q = 0

    with tile.TileContext(nc) as tc:
        with (
            tc.tile_pool(name="const", bufs=1) as const_pool,
            tc.tile_pool(name="feats", bufs=6) as feat_pool,
            tc.tile_pool(name="tfeat", bufs=4) as tfeat_pool,
            tc.tile_pool(name="outs", bufs=4) as out_pool,
            tc.tile_pool(name="tpsum", bufs=4, space="PSUM") as tpsum_pool,
            tc.tile_pool(name="opsum", bufs=3, space="PSUM") as opsum_pool,
        ):
            # Constants
            w_tile = const_pool.tile([P, 3, D], bf16)
            nc.sync.dma_start(w_tile[:], w_dram[:])
            b_tile = const_pool.tile([P, 1], f32)
            nc.sync.dma_start(b_tile[:], b_dram[:, None])
            if not TRANSPOSE_GATHER:
                identity = const_pool.tile([P, P], bf16)
                nc.sync.dma_start(identity[:], ident_dram[:])

            for blk in range(n_blocks):
                src_hi, dst_hi = block_cls[blk]

                idx_blk = feat_pool.tile([P, 2 * idx_words], i16, tag="idx")
                nc.sync.dma_start(
                    idx_blk[:],
                    idx_dram[:, (2 * blk) * idx_words : (2 * blk + 2) * idx_words],
                )
                sidx = idx_blk[:, :idx_words]
                didx = idx_blk[:, idx_words:]

                sb_e = feat_pool.tile([P, block], bf16, tag="sb_e")
                nc.sync.dma_start(sb_e[:], edge_featT[blk])

                if TRANSPOSE_GATHER:
                    # feature-major gathers via the XBAR; chunked so each
                    # single_packet stream stays within the SDMA
                    # 64-descriptor-per-packet ceiling
                    sb_s = feat_pool.tile([P, 1, block], bf16, tag="sb_s")
                    sb_d = feat_pool.tile([P, 1, block], bf16, tag="sb_d")
                    for (sb, idx_w, hi) in (
                        (sb_s, sidx, src_hi),
                        (sb_d, didx, dst_hi),
                    ):
                        qn = gather_q % N_QUEUES
                        gather_q += 1
                        j0 = 0
                        while j0 < block:
                            chunk = min(GATHER_CHUNK, block - j0)
                            nc.gpsimd.dma_gather(
                                out_ap=sb[:, :, j0 : j0 + chunk],
                                in_ap=nf_hi if hi else nf_lo,
                                idxs_ap=idx_w[:, j0 // 16 : (j0 + chunk) // 16],
                                num_idxs=chunk,
                                num_idxs_reg=chunk,
                                elem_size=D,
                                transpose=True,
                                single_packet=SINGLE_PACKET,
                                queue_num=qn,
                            )
                            j0 += chunk
                    rhs_s = lambda sl: sb_s[:, 0, sl]
                    rhs_d = lambda sl: sb_d[:, 0, sl]
                else:
                    # edge-major gathers (partition = edge%128), then PE
                    # transposes per 128-edge chunk with ACT PSUM evac.
                    # src+dst idx words are adjacent in idx_all and their
                    # tiles adjacent in sb_sd, so same-half blocks gather
                    # both operands in ONE 4096-idx call.
                    sb_sd = feat_pool.tile([P, 2 * c, D], bf16, tag="sb_sd")
                    sb_sr = sb_sd[:, :c, :]
                    sb_dr = sb_sd[:, c:, :]
                    for (sb, idx_w, hi) in (
                        (sb_sr, sidx, src_hi),
                        (sb_dr, didx, dst_hi),
                    ):
                        j0 = 0
                        while j0 < block:
                            chunk = min(GATHER_CHUNK_NT, block - j0)
                            nc.gpsimd.dma_gather(
                                out_ap=sb[:, j0 // P : (j0 + chunk) // P, :],
                                in_ap=nf_hi if hi else nf_lo,
                                idxs_ap=idx_w[:, j0 // 16 : (j0 + chunk) // 16],
                                num_idxs=chunk,
                                num_idxs_reg=chunk,
                                elem_size=D,
                                transpose=False,
                                single_packet=SINGLE_PACKET,
                                queue_num=gather_q % N_QUEUES,
                            )
                            gather_q += 1
                            j0 += chunk

                sb_out = out_pool.tile([P, block], bf16, tag="sb_out")

                for g in range(n_groups):
                    sl = slice(g * 512, (g + 1) * 512)
                    if not TRANSPOSE_GATHER:
                        # both operands' transposes share one PSUM bank;
                        # single DVE evac per group
                        ps_t = tpsum_pool.tile([P, 2, 512], bf16, tag="tp")
                        for oi in range(2):
                            for cl in range(4):
                                ch = oi * c + 4 * g + cl
                                nc.tensor.transpose(
                                    ps_t[:, oi, cl * P : (cl + 1) * P],
                                    sb_sd[:, ch, :],
                                    identity[:],
                                )
                        # evac via ACT (own SBUF ports): DVE copies would
                        # contend with the Q7 descriptor-generation port
                        sb_t = tfeat_pool.tile([P, 2, 512], bf16, tag="t")
                        nc.scalar.activation(
                            sb_t[:], ps_t[:], mybir.ActivationFunctionType.Copy
                        )
                        rhs_s = lambda _sl, t=sb_t: t[:, 0, :]
                        rhs_d = lambda _sl, t=sb_t: t[:, 1, :]

                    ps_o = opsum_pool.tile([P, 512], f32, tag="op")
                    nc.tensor.matmul(
                        ps_o[:], lhsT=w_tile[:, 0, :], rhs=rhs_s(sl),
                        start=True, stop=False,
                    )
                    nc.tensor.matmul(
                        ps_o[:], lhsT=w_tile[:, 1, :], rhs=rhs_d(sl),
                        start=False, stop=False,
                    )
                    nc.tensor.matmul(
                        ps_o[:], lhsT=w_tile[:, 2, :], rhs=sb_e[:, sl],
                        start=False, stop=True,
                    )
                    nc.scalar.activation(
                        sb_out[:, sl],
                        ps_o[:],
                        mybir.ActivationFunctionType.Relu,
                        bias=b_tile[:],
                    )

                nc.sync.dma_start(outT[blk], sb_out[:])

    nc.compile()
    return nc


_PROGRAM_CACHE = {}


def _get_program(class_blocks=CLASS_BLOCKS):
    key = tuple(class_blocks)
    if key not in _PROGRAM_CACHE:
        _PROGRAM_CACHE[key] = build_program(class_blocks=key)
    return _PROGRAM_CACHE[key]


def shard_core(edge_feat_c, src_c, dst_c, class_blocks, half=HALF, c=C, n_nodes=N_NODES):
    """Classify/permute/pad one core's edges. Returns device arrays + the
    permutation metadata for output recovery."""
    block = P * c
    idx_words = block // 16
    n_blocks = sum(class_blocks)
    e_pad = block * n_blocks
    cls = (src_c >= half).astype(np.int64) * 2 + (dst_c >= half).astype(np.int64)
    order = np.argsort(cls, kind="stable")
    counts = np.bincount(cls, minlength=4)
    seg_off = np.concatenate([[0], np.cumsum(np.asarray(class_blocks) * block)])

    ef_pad = np.zeros((e_pad, D), np.float32)
    # pad slots get distinct gather indices (not all-0) so the SDMA engines
    # don't hammer one HBM line; cap within both table halves
    cap = min(half, n_nodes - half)
    spread = (np.arange(e_pad) % cap).astype(np.int16)
    s_pad = spread.copy()
    d_pad = spread.copy()
    src_reb = np.where(src_c >= half, src_c - half, src_c).astype(np.int16)
    dst_reb = np.where(dst_c >= half, dst_c - half, dst_c).astype(np.int16)

    pos = 0
    for k in range(4):
        sel = order[pos : pos + counts[k]]
        off = seg_off[k]
        ef_pad[off : off + counts[k]] = edge_feat_c[sel]
        s_pad[off : off + counts[k]] = src_reb[sel]
        d_pad[off : off + counts[k]] = dst_reb[sel]
        pos += counts[k]

    # device edge_feat layout: ef_T[b, f, j] = padded edge (b*block + j), feat f
    ef_dev = np.ascontiguousarray(
        ef_pad.reshape(n_blocks, block, D).transpose(0, 2, 1)
    ).astype(BF16)

    # idx wrap: word s of partition q%16 = index i = s*16 + (q%16), and the
    # 16-partition block is replicated for each of the 8 GPSIMD Q7 cores
    def wrap(v):
        w16 = v.reshape(n_blocks, idx_words, 16).transpose(0, 2, 1)  # [b,16,words]
        return np.ascontiguousarray(np.tile(w16, (1, P // 16, 1)))

    sw, dw = wrap(s_pad), wrap(d_pad)
    # [128, n_blocks*2*idx_words] interleaved (src, dst) per block
    idx_all = np.ascontiguousarray(
        np.stack([sw, dw], axis=1).transpose(2, 0, 1, 3).reshape(P, -1)
    )
    return ef_dev, idx_all, order, counts, seg_off


def unshard_core(outT_core, order, counts, seg_off):
    """[n_blocks, 128, block] bf16 kernel output -> [E, D] f32 in original
    edge order."""
    o = np.asarray(outT_core).astype(np.float32)
    n_blocks = o.shape[0]
    oT = o.transpose(0, 2, 1).reshape(n_blocks * o.shape[2], D)  # row i = padded edge i
    out_c = np.empty((len(order), D), np.float32)
    pos = 0
    for k in range(4):
        sel = order[pos : pos + counts[k]]
        out_c[sel] = oT[seg_off[k] : seg_off[k] + counts[k]]
        pos += counts[k]
    return out_c


def _needed_blocks(counts, c=C):
    block = P * c
    return tuple(int(-(-int(n) // block)) if n else 1 for n in counts)


def kernel(node_feat, edge_feat, W, b, src, dst):
    out, _ = kernel_with_results(node_feat, edge_feat, W, b, src, dst)
    return out


def kernel_with_results(node_feat, edge_feat, W, b, src, dst, **spmd_kwargs):
    from concourse.bass_utils import run_bass_kernel_spmd

    node_feat = np.ascontiguousarray(np.asarray(node_feat, dtype=np.float32))
    edge_feat = np.ascontiguousarray(np.asarray(edge_feat, dtype=np.float32))
    W = np.ascontiguousarray(np.asarray(W, dtype=np.float32))
    b = np.ascontiguousarray(np.asarray(b, dtype=np.float32))
    src = np.ascontiguousarray(np.asarray(src, dtype=np.int32))
    dst = np.ascontiguousarray(np.asarray(dst, dtype=np.int32))

    node_bf = np.ascontiguousarray(node_feat.astype(BF16))
    # W_r[f, k, j] = W[k*128 + f, j]
    w_dev = np.ascontiguousarray(
        W.reshape(3, P, D).transpose(1, 0, 2)
    ).astype(BF16)

    # default class capacities; grow (with a rebuild) if any core overflows
    class_blocks = list(CLASS_BLOCKS)
    per_core = []
    for i in range(N_CORES):
        lo = i * E_LOC
        sc, dc = src[lo : lo + E_LOC], dst[lo : lo + E_LOC]
        cls = (sc >= HALF).astype(np.int64) * 2 + (dc >= HALF).astype(np.int64)
        counts = np.bincount(cls, minlength=4)
        need = _needed_blocks(counts)
        class_blocks = [max(a, b_) for a, b_ in zip(class_blocks, need)]
        per_core.append((sc, dc))
    class_blocks = tuple(class_blocks)

    nc = _get_program(class_blocks)

    in_maps = []
    metas = []
    for i in range(N_CORES):
        lo = i * E_LOC
        sc, dc = per_core[i]
        ef_dev, idx_all, order, counts, seg_off = shard_core(
            edge_feat[lo : lo + E_LOC], sc, dc, class_blocks
        )
        metas.append((order, counts, seg_off))
        im = {
            "node_feat": node_bf,
            "edge_featT": ef_dev,
            "W": w_dev,
            "b": b,
            "idx_all": idx_all,
        }
        if not TRANSPOSE_GATHER:
            im["ident"] = np.eye(P, dtype=BF16)
        in_maps.append(im)

    res = run_bass_kernel_spmd(nc, in_maps, list(range(N_CORES)), **spmd_kwargs)
    outs = []
    for i in range(N_CORES):
        order, counts, seg_off = metas[i]
        outs.append(unshard_core(res.results[i]["outT"], order, counts, seg_off))
    return np.concatenate(outs, axis=0), res


# revision 27
# speedup vs baseline: 1.0526x; 1.0526x over previous
"""AggBondModule kernel for Trainium2 (8 NeuronCores, SPMD edge-sharding).

out[e] = relu(concat(node_feat[src[e]], node_feat[dst[e]], edge_feat[e]) @ W + b)

Per core (50000 edges), all feature data is bf16 on device. Node rows are
fetched with the ANT `dma_gather` instruction in TRANSPOSE mode: each
2048-index gather lands feature-major ([128 feat partitions, 2048 edges])
directly, so no PE transposes or PSUM round-trips are needed. The gathers
are round-robined over the 4 SWDGE queues so all four Q7 core pairs
generate descriptors concurrently (descriptor generation on a single pair
is the baseline bottleneck).

Since gather indices are int16 (< 32768) and there are 50000 nodes, the
node table is addressed in two halves and the host pre-sorts each core's
edges into four classes by (src < 32768, dst < 32768); each 2048-edge
block gathers from a fixed half per operand with rebased indices. Pad
slots use index 0. The host un-permutes the output.

edge_feat is host-transposed per block ([128, 2048] bf16) so it DMAs
straight into feature-major tiles. Compute per 512-edge group: three bf16
matmuls (W chunks stationary, 512-edge moving dim) accumulate into PSUM;
ACT applies bias+ReLU on the PSUM->SBUF copy, emitting bf16. Output is
stored transposed per block and rearranged on the host.
"""

import sys

import numpy as np

sys.path.insert(0, "/opt/trn_rl_repo")

import ml_dtypes

BF16 = np.dtype(ml_dtypes.bfloat16)

P = 128
D = 128  # node/edge feature dim; also output dim
N_NODES = 50000
HALF = 32768  # int16-addressable node-table split
E_TOTAL = 400000
N_CORES = 8
C = 16  # 128-edge chunks per block
BLOCK = P * C  # 2048
# blocks per (src-half, dst-half) class: AA, AB, BA, BB
CLASS_BLOCKS = (11, 6, 6, 3)
E_LOC = E_TOTAL // N_CORES  # 50000
N_QUEUES = 4  # SWDGE queues (each maps to its own Q7 core pair)
SINGLE_PACKET = False
GATHER_CHUNK = 896  # idxs per dma_gather call in transpose mode
GATHER_CHUNK_NT = 2048  # idxs per dma_gather call in non-transpose mode
# Transpose-mode gathers land feature-major (no PE transposes) but their
# XBAR spray streams corrupt each other when gathers run concurrently on
# multiple SWDGE queues. Non-transpose gathers land edge-major (PE
# transposes + DVE evac needed) but are XBAR-free, so 4-queue concurrent
# descriptor generation is safe.
TRANSPOSE_GATHER = False


def build_program(
    n_nodes=N_NODES,
    half=HALF,
    c=C,
    class_blocks=CLASS_BLOCKS,
    num_devices=N_CORES,
):
    from concourse import bacc, mybir, tile

    block = P * c
    n_blocks = sum(class_blocks)
    n_groups = block // 512  # 512-edge matmul groups per block
    idx_words = block // 16
    f32 = mybir.dt.float32
    bf16 = mybir.dt.bfloat16
    i16 = mybir.dt.int16

    nc = bacc.Bacc(
        "TRN2",
        target_bir_lowering=False,
        debug=False,
        num_devices=num_devices,
        num_swdge_queues=N_QUEUES,
    )

    node_feat = nc.dram_tensor("node_feat", [n_nodes, D], bf16, kind="ExternalInput").ap()
    # edge_featT[b, f, j] = edge_feat[edge (b*block + j), f]  (host-prepared)
    edge_featT = nc.dram_tensor(
        "edge_featT", [n_blocks, P, block], bf16, kind="ExternalInput"
    ).ap()
    # W_r[f, k, j] = W[k*128 + f, j]  (host-prepared)
    w_dram = nc.dram_tensor("W", [P, 3, D], bf16, kind="ExternalInput").ap()
    b_dram = nc.dram_tensor("b", [D], f32, kind="ExternalInput").ap()
    # all blocks' wrapped indices, preloaded in one DMA:
    # idx_all[p, ((b*2+which)*idx_words + w)], which: 0=src, 1=dst
    idx_dram = nc.dram_tensor(
        "idx_all", [P, n_blocks * 2 * idx_words], i16, kind="ExternalInput"
    ).ap()
    if not TRANSPOSE_GATHER:
        ident_dram = nc.dram_tensor("ident", [P, P], bf16, kind="ExternalInput").ap()
    outT = nc.dram_tensor("outT", [n_blocks, P, block], bf16, kind="ExternalOutput").ap()

    nf_lo = node_feat[:half, :]
    nf_hi = node_feat[half:, :]

    # block -> (src_half_hi, dst_half_hi)
    block_cls = []
    for cls_i, nb in enumerate(class_blocks):
        block_cls += [(cls_i >> 1 & 1, cls_i & 1)] * nb

    gather_q = 0

    with tile.TileContext(nc) as tc:
        with (
            tc.tile_pool(name="const", bufs=1) as const_pool,
            tc.tile_pool(name="feats", bufs=6) as feat_pool,
            tc.tile_pool(name="tfeat", bufs=4) as tfeat_pool,
            tc.tile_pool(name="outs", bufs=4) as out_pool,
            tc.tile_pool(name="tpsum", bufs=4, space="PSUM") as tpsum_pool,
            tc.tile_pool(name="opsum", bufs=4, space="PSUM") as opsum_pool,
        ):
            # Constants
            w_tile = const_pool.tile([P, 3, D], bf16)
            nc.sync.dma_start(w_tile[:], w_dram[:])
            b_tile = const_pool.tile([P, 1], f32)
            nc.sync.dma_start(b_tile[:], b_dram[:, None])
            idx_all = const_pool.tile([P, n_blocks * 2 * idx_words], i16)
            nc.sync.dma_start(idx_all[:], idx_dram[:])
            if not TRANSPOSE_GATHER:
                identity = const_pool.tile([P, P], bf16)
                nc.sync.dma_start(identity[:], ident_dram[:])

            for blk in range(n_blocks):
                src_hi, dst_hi = block_cls[blk]

                sidx = idx_all[:, (2 * blk) * idx_words : (2 * blk + 1) * idx_words]
                didx = idx_all[:, (2 * blk + 1) * idx_words : (2 * blk + 2) * idx_words]

                sb_e = feat_pool.tile([P, block], bf16, tag="sb_e")
                nc.sync.dma_start(sb_e[:], edge_featT[blk])

                if TRANSPOSE_GATHER:
                    # feature-major gathers via the XBAR; chunked so each
                    # single_packet stream stays within the SDMA
                    # 64-descriptor-per-packet ceiling
                    sb_s = feat_pool.tile([P, 1, block], bf16, tag="sb_s")
                    sb_d = feat_pool.tile([P, 1, block], bf16, tag="sb_d")
                    for (sb, idx_w, hi) in (
                        (sb_s, sidx, src_hi),
                        (sb_d, didx, dst_hi),
                    ):
                        qn = gather_q % N_QUEUES
                        gather_q += 1
                        j0 = 0
                        while j0 < block:
                            chunk = min(GATHER_CHUNK, block - j0)
                            nc.gpsimd.dma_gather(
                                out_ap=sb[:, :, j0 : j0 + chunk],
                                in_ap=nf_hi if hi else nf_lo,
                                idxs_ap=idx_w[:, j0 // 16 : (j0 + chunk) // 16],
                                num_idxs=chunk,
                                num_idxs_reg=chunk,
                                elem_size=D,
                                transpose=True,
                                single_packet=SINGLE_PACKET,
                                queue_num=qn,
                            )
                            j0 += chunk
                    rhs_s = lambda sl: sb_s[:, 0, sl]
                    rhs_d = lambda sl: sb_d[:, 0, sl]
                else:
                    # edge-major gathers (partition = edge%128), then PE
                    # transposes per 128-edge chunk with ACT PSUM evac.
                    # src+dst idx words are adjacent in idx_all and their
                    # tiles adjacent in sb_sd, so same-half blocks gather
                    # both operands in ONE 4096-idx call.
                    sb_sd = feat_pool.tile([P, 2 * c, D], bf16, tag="sb_sd")
                    sb_sr = sb_sd[:, :c, :]
                    sb_dr = sb_sd[:, c:, :]
                    for (sb, idx_w, hi) in (
                        (sb_sr, sidx, src_hi),
                        (sb_dr, didx, dst_hi),
                    ):
                        j0 = 0
                        while j0 < block:
                            chunk = min(GATHER_CHUNK_NT, block - j0)
                            nc.gpsimd.dma_gather(
                                out_ap=sb[:, j0 // P : (j0 + chunk) // P, :],
                                in_ap=nf_hi if hi else nf_lo,
                                idxs_ap=idx_w[:, j0 // 16 : (j0 + chunk) // 16],
                                num_idxs=chunk,
                                num_idxs_reg=chunk,
                                elem_size=D,
                                transpose=False,
                                single_packet=SINGLE_PACKET,
                                queue_num=gather_q % N_QUEUES,
                            )
                            gather_q += 1
                            j0 += chunk

                sb_out = out_pool.tile([P, block], bf16, tag="sb_out")

                for g in range(n_groups):
                    sl = slice(g * 512, (g + 1) * 512)
                    if not TRANSPOSE_GATHER:
                        # both operands' transposes share one PSUM bank;
                        # single DVE evac per group
                        ps_t = tpsum_pool.tile([P, 2, 512], bf16, tag="tp")
                        for oi in range(2):
                            for cl in range(4):
                                ch = oi * c + 4 * g + cl
                                nc.tensor.transpose(
                                    ps_t[:, oi, cl * P : (cl + 1) * P],
                                    sb_sd[:, ch, :],
                                    identity[:],
                                )
                        # evac via ACT (own SBUF ports): DVE copies would
                        # contend with the Q7 descriptor-generation port
                        sb_t = tfeat_pool.tile([P, 2, 512], bf16, tag="t")
                        nc.scalar.activation(
                            sb_t[:], ps_t[:], mybir.ActivationFunctionType.Copy
                        )
                        rhs_s = lambda _sl, t=sb_t: t[:, 0, :]
                        rhs_d = lambda _sl, t=sb_t: t[:, 1, :]

                    ps_o = opsum_pool.tile([P, 512], f32, tag="op")
                    nc.tensor.matmul(
                        ps_o[:], lhsT=w_tile[:, 0, :], rhs=rhs_s(sl),
                        start=True, stop=False,
                    )
                    nc.tensor.matmul(
                        ps_o[:], lhsT=w_tile[:, 1, :], rhs=rhs_d(sl),
                        start=False, stop=False,
                    )
                    nc.tensor.matmul(
                        ps_o[:], lhsT=w_tile[:, 2, :], rhs=sb_e[:, sl],
                        start=False, stop=True,
                    )
                    nc.scalar.activation(
                        sb_out[:, sl],
                        ps_o[:],
                        mybir.ActivationFunctionType.Relu,
                        bias=b_tile[:],
                    )

                nc.sync.dma_start(outT[blk], sb_out[:])

    nc.compile()
    return nc


_PROGRAM_CACHE = {}


def _get_program(class_blocks=CLASS_BLOCKS):
    key = tuple(class_blocks)
    if key not in _PROGRAM_CACHE:
        _PROGRAM_CACHE[key] = build_program(class_blocks=key)
    return _PROGRAM_CACHE[key]


def shard_core(edge_feat_c, src_c, dst_c, class_blocks, half=HALF, c=C, n_nodes=N_NODES):
    """Classify/permute/pad one core's edges. Returns device arrays + the
    permutation metadata for output recovery."""
    block = P * c
    idx_words = block // 16
    n_blocks = sum(class_blocks)
    e_pad = block * n_blocks
    cls = (src_c >= half).astype(np.int64) * 2 + (dst_c >= half).astype(np.int64)
    order = np.argsort(cls, kind="stable")
    counts = np.bincount(cls, minlength=4)
    seg_off = np.concatenate([[0], np.cumsum(np.asarray(class_blocks) * block)])

    ef_pad = np.zeros((e_pad, D), np.float32)
    # pad slots get distinct gather indices (not all-0) so the SDMA engines
    # don't hammer one HBM line; cap within both table halves
    cap = min(half, n_nodes - half)
    spread = (np.arange(e_pad) % cap).astype(np.int16)
    s_pad = spread.copy()
    d_pad = spread.copy()
    src_reb = np.where(src_c >= half, src_c - half, src_c).astype(np.int16)
    dst_reb = np.where(dst_c >= half, dst_c - half, dst_c).astype(np.int16)

    pos = 0
    for k in range(4):
        sel = order[pos : pos + counts[k]]
        off = seg_off[k]
        ef_pad[off : off + counts[k]] = edge_feat_c[sel]
        s_pad[off : off + counts[k]] = src_reb[sel]
        d_pad[off : off + counts[k]] = dst_reb[sel]
        pos += counts[k]

    # device edge_feat layout: ef_T[b, f, j] = padded edge (b*block + j), feat f
    ef_dev = np.ascontiguousarray(
        ef_pad.reshape(n_blocks, block, D).transpose(0, 2, 1)
    ).astype(BF16)

    # idx wrap: word s of partition q%16 = index i = s*16 + (q%16), and the
    # 16-partition block is replicated for each of the 8 GPSIMD Q7 cores
    def wrap(v):
        w16 = v.reshape(n_blocks, idx_words, 16).transpose(0, 2, 1)  # [b,16,words]
        return np.ascontiguousarray(np.tile(w16, (1, P // 16, 1)))

    sw, dw = wrap(s_pad), wrap(d_pad)
    # [128, n_blocks*2*idx_words] interleaved (src, dst) per block
    idx_all = np.ascontiguousarray(
        np.stack([sw, dw], axis=1).transpose(2, 0, 1, 3).reshape(P, -1)
    )
    return ef_dev, idx_all, order, counts, seg_off


def unshard_core(outT_core, order, counts, seg_off):
    """[n_blocks, 128, block] bf16 kernel output -> [E, D] f32 in original
    edge order."""
    o = np.asarray(outT_core).astype(np.float32)
    n_blocks = o.shape[0]
    oT = o.transpose(0, 2, 1).reshape(n_blocks * o.shape[2], D)  # row i = padded edge i
    out_c = np.empty((len(order), D), np.float32)
    pos = 0
    for k in range(4):
        sel = order[pos : pos + counts[k]]
        out_c[sel] = oT[seg_off[k] : seg_off[k] + counts[k]]
        pos += counts[k]
    return out_c


def _needed_blocks(counts, c=C):
    block = P * c
    return tuple(int(-(-int(n) // block)) if n else 1 for n in counts)


def kernel(node_feat, edge_feat, W, b, src, dst):
    out, _ = kernel_with_results(node_feat, edge_feat, W, b, src, dst)
    return out


def kernel_with_results(node_feat, edge_feat, W, b, src, dst, **spmd_kwargs):
    from concourse.bass_utils import run_bass_kernel_spmd

    node_feat = np.ascontiguousarray(np.asarray(node_feat, dtype=np.float32))
    edge_feat = np.ascontiguousarray(np.asarray(edge_feat, dtype=np.float32))
    W = np.ascontiguousarray(np.asarray(W, dtype=np.float32))
    b = np.ascontiguousarray(np.asarray(b, dtype=np.float32))
    src = np.ascontiguousarray(np.asarray(src, dtype=np.int32))
    dst = np.ascontiguousarray(np.asarray(dst, dtype=np.int32))

    node_bf = np.ascontiguousarray(node_feat.astype(BF16))
    # W_r[f, k, j] = W[k*128 + f, j]
    w_dev = np.ascontiguousarray(
        W.reshape(3, P, D).transpose(1, 0, 2)
    ).astype(BF16)

    # default class capacities; grow (with a rebuild) if any core overflows
    class_blocks = list(CLASS_BLOCKS)
    per_core = []
    for i in range(N_CORES):
        lo = i * E_LOC
        sc, dc = src[lo : lo + E_LOC], dst[lo : lo + E_LOC]
        cls = (sc >= HALF).astype(np.int64) * 2 + (dc >= HALF).astype(np.int64)
        counts = np.bincount(cls, minlength=4)
        need = _needed_blocks(counts)
        class_blocks = [max(a, b_) for a, b_ in zip(class_blocks, need)]
        per_core.append((sc, dc))
    class_blocks = tuple(class_blocks)

    nc = _get_program(class_blocks)

    in_maps = []
    metas = []
    for i in range(N_CORES):
        lo = i * E_LOC
        sc, dc = per_core[i]
        ef_dev, idx_all, order, counts, seg_off = shard_core(
            edge_feat[lo : lo + E_LOC], sc, dc, class_blocks
        )
        metas.append((order, counts, seg_off))
        im = {
            "node_feat": node_bf,
            "edge_featT": ef_dev,
            "W": w_dev,
            "b": b,
            "idx_all": idx_all,
        }
        if not TRANSPOSE_GATHER:
            im["ident"] = np.eye(P, dtype=BF16)
        in_maps.append(im)

    res = run_bass_kernel_spmd(nc, in_maps, list(range(N_CORES)), **spmd_kwargs)
    outs = []
    for i in range(N_CORES):
        order, counts, seg_off = metas[i]
        outs.append(unshard_core(res.results[i]["outT"], order, counts, seg_off))
    return np.concatenate(outs, axis=0), res
